# revision 1
# baseline (speedup 1.0000x reference)
"""Trainium2 Bass kernel for nn_DegreeEmbeddingNetwork (gnn_message_passing).

Strategy (8 NeuronCores, SPMD single program):
  - The reference collapses massively: node features are a constant broadcast
    (s0 = lin_w + lin_b) and the l=1 node block is structurally zero, so the
    whole per-edge computation is
        h   = scalars @ rad_w1                  (radial MLP layer 1)
        h2  = silu(LN(h))                       (per-edge layernorm over 64)
        q   = h2 @ B  (+ c)                     (B folds rad_w2 x TP x proj)
        deg = [a0*q0 | outer(q1, a1)]           (160 wide)
        out = scatter_add(deg by dst) / sqrt(32)
  - Host folds all small weight matrices into W1c (mean-centered: LN mean
    subtraction is linear so it folds into rad_w1) and B; rad_off/proj_b0
    contributions are exact rank-1 host-side corrections.
  - Edges are sorted by destination node on the host; core k owns nodes
    [k*NPC, (k+1)*NPC), sees only its own edges, and scatter-adds locally via
    one-hot matmuls into 128-node windows (no collectives needed; host
    concatenates the 8 node shards).
  - Per 128-edge tile on device:
      MM1 (edge-major, lhsT = X.T tile)  -> Hc (centered h) in PSUM
      ACT square + DVE reduce            -> ssq -> rstd (batched per 8 tiles)
      DVE normalize (Hc * rstd)          -> N4
      ACT silu                           -> H2
      PE transpose + DMA                 -> H2.T in SBUF
      MM2 (lhsT = H2.T, rhs = B)         -> Q (edge-major) in PSUM
      DVE deg build (a0*q0, q1 x a1)     -> deg
      POOL/DVE onehot (iota == off)      -> oh
      scatter matmul (lhsT=oh, rhs=deg)  -> window accumulator in PSUM
"""

import math
import sys

sys.path.insert(0, "/opt/trn_rl_repo")

import numpy as np

import concourse.bacc as bacc
import concourse.tile as tile
from concourse import mybir
from concourse.bass_utils import run_bass_kernel_spmd

F32 = mybir.dt.float32
F32R = mybir.dt.float32r
BF16 = mybir.dt.bfloat16

N_CORES = 8
MUL0, MUL1 = 64, 32
D_EMB = 160
RAD_HID = 64
AVG_AGG = 32.0
LN_EPS = 1e-5
WIN = 128          # nodes per scatter window
SUP = 4            # tiles per supertile (elementwise batch)
GRP = 8            # tiles per stats group
SG = 2             # groups per super-group (sqrt batch)
DEG_W = 160        # deg width (pad to 256 for f32r scatter)

CONFIG = {
    "mm1_dt": "f32",    # lhsT=X.T tile [64,128], rhs=W1c [64,64]
    "mm2_dt": "f32",    # lhsT=H2.T [64,128], rhs=B [64,96]
    "scat_dt": "f32",   # lhsT=onehot [128,128], rhs=deg [128, DEG_W]
    "trans_dt": "f32",   # PE transpose dtype for H2
    "onehot_engine": "gpsimd",  # "gpsimd" | "vector"
}

_PROGRAM_CACHE = {}
_LAST_IN_MAPS = None


def _mmdt(name):
    return {"f32r": F32R, "f32": F32, "bf16": BF16}[name]


def _mm_ap(ap, dtname):
    dt = _mmdt(dtname)
    if dt == F32:
        return ap
    return ap.bitcast(dt)


def build_program(C, NW, TPW, NT, general_affine):
    """Build the SPMD Bass program. C = padded edges/core, NW windows of 128
    nodes, TPW tiles per window, NT = total tiles (multiple of SG*GRP)."""
    MMDT = _mmdt(CONFIG["mm1_dt"])
    SCDT = _mmdt(CONFIG["scat_dt"])
    nc = bacc.Bacc("TRN2", target_bir_lowering=False, debug=False,
                   num_devices=N_CORES)

    xt_d = nc.dram_tensor("xt", [64, C], MMDT, kind="ExternalInput").ap()
    aux_d = nc.dram_tensor("aux", [NT // (SG * GRP), 128, SG * GRP * 5], F32,
                           kind="ExternalInput").ap()
    w1_d = nc.dram_tensor("w1c", [64, 64], MMDT, kind="ExternalInput").ap()
    b_d = nc.dram_tensor("bmat", [128, 96], MMDT, kind="ExternalInput").ap()
    iota_d = nc.dram_tensor("iota", [128, WIN], F32, kind="ExternalInput").ap()
    ident_d = nc.dram_tensor("ident", [128, 128], MMDT, kind="ExternalInput").ap()
    if general_affine:
        gb_d = nc.dram_tensor("gbc", [128, 128], F32, kind="ExternalInput").ap()
    out_d = nc.dram_tensor("out", [NW * 128, D_EMB], F32,
                           kind="ExternalOutput").ap()

    # super-group = SG groups of GRP tiles; stats (sqrt/recip) batched per
    # super-group so the ACT table only flips twice per SG*GRP tiles.
    SGT = SG * GRP                       # tiles per super-group
    assert NT % SGT == 0

    with tile.TileContext(nc) as tc:
        with (
            tc.tile_pool(name="consts", bufs=1) as cpool,
            tc.tile_pool(name="xt", bufs=3) as xt_pool,
            tc.tile_pool(name="aux", bufs=3) as aux_pool,
            tc.tile_pool(name="sq", bufs=4) as sq_pool,
            tc.tile_pool(name="stats", bufs=3) as st_pool,
            tc.tile_pool(name="n4", bufs=4) as n4_pool,
            tc.tile_pool(name="h2", bufs=4) as h2_pool,
            tc.tile_pool(name="h2t", bufs=6) as h2t_pool,
            tc.tile_pool(name="oh", bufs=8) as oh_pool,
            tc.tile_pool(name="deg", bufs=1) as deg_pool,
            tc.tile_pool(name="flush", bufs=3) as fl_pool,
            tc.tile_pool(name="psH", bufs=3, space="PSUM") as psH,
            tc.tile_pool(name="psQT", bufs=2, space="PSUM") as psQT,
            tc.tile_pool(name="psT", bufs=2, space="PSUM") as psT,
            tc.tile_pool(name="psA", bufs=1, space="PSUM") as psA,
        ):
            w1_sb = cpool.tile([64, 64], MMDT)
            nc.sync.dma_start(w1_sb[:], w1_d[:])
            b_sb = cpool.tile([128, 96], MMDT)
            nc.sync.dma_start(b_sb[:], b_d[:])
            iota_sb = cpool.tile([128, WIN], F32)
            nc.sync.dma_start(iota_sb[:], iota_d[:])
            ident_sb = cpool.tile([128, 128], MMDT)
            nc.sync.dma_start(ident_sb[:], ident_d[:])
            if general_affine:
                gb_sb = cpool.tile([128, 128], F32)
                nc.sync.dma_start(gb_sb[:], gb_d[:])
            eps_sb = cpool.tile([128, 1], F32)
            nc.vector.memset(eps_sb[:], LN_EPS)

            # fixed deg buffers: pad columns zeroed once, never rewritten
            deg_bufs = []
            for i in range(2):
                d = deg_pool.tile([128, SUP * DEG_W], SCDT, tag=f"deg{i}")
                nc.vector.memset(d[:].bitcast(F32), 0.0)
                deg_bufs.append(d)

            def winof(nt):
                return min(nt // TPW, NW - 1)

            acc = None
            acc_win = -1

            for sg in range(NT // SGT):
                # ---- pass 1: MM1 + square + reduce for SGT tiles ----
                ssq = st_pool.tile([128, SGT], F32, tag="ssq")
                xtg = xt_pool.tile([64, SGT * 128], MMDT)
                nc.sync.dma_start(xtg[:], xt_d[:, sg * SGT * 128:(sg + 1) * SGT * 128])
                auxg = aux_pool.tile([128, SGT * 5], F32)
                nc.sync.dma_start(auxg[:], aux_d[sg])
                h_of_g = []
                for gi in range(SG):
                    # one PSUM bank holds H for a whole group (GRP*64 cols)
                    H8 = psH.tile([128, GRP * 64], F32)
                    h_of_g.append(H8)
                    for si in range(GRP // SUP):
                        nt0 = sg * SGT + gi * GRP + si * SUP
                        loc0 = (gi * GRP + si * SUP) * 128
                        for t in range(SUP):
                            nc.tensor.matmul(
                                H8[:, (si * SUP + t) * 64:(si * SUP + t + 1) * 64],
                                xtg[:, loc0 + t * 128:loc0 + (t + 1) * 128],
                                w1_sb[:],
                                start=True, stop=True)
                        sq4 = sq_pool.tile([128, SUP * 64], F32)
                        nc.scalar.activation(
                            sq4[:], H8[:, si * SUP * 64:(si + 1) * SUP * 64],
                            mybir.ActivationFunctionType.Square)
                        nc.vector.tensor_reduce(
                            ssq[:, gi * GRP + si * SUP:gi * GRP + (si + 1) * SUP],
                            sq4[:].rearrange("p (t f) -> p t f", f=64),
                            axis=mybir.AxisListType.X, op=mybir.AluOpType.add)

                # ---- stats for the whole super-group ----
                std = st_pool.tile([128, SGT], F32, tag="std")
                nc.scalar.activation(std[:], ssq[:],
                                     mybir.ActivationFunctionType.Sqrt,
                                     bias=eps_sb[:], scale=1.0 / 64.0)
                rstd = st_pool.tile([128, SGT], F32, tag="rstd")
                nc.vector.reciprocal(rstd[:], std[:])

                # ---- pass 2 ----
                for gi in range(SG):
                    H8 = h_of_g[gi]
                    for si in range(GRP // SUP):
                        s_loc = gi * GRP + si * SUP          # tile offset in sg
                        nt0 = sg * SGT + s_loc
                        a3 = (auxg[:, s_loc * 5:(s_loc + SUP) * 5]
                              .rearrange("p (t f) -> p t f", f=5))

                        H4 = H8[:, si * SUP * 64:(si + 1) * SUP * 64]
                        N4 = n4_pool.tile([128, SUP * 64], F32)
                        rex = (rstd[:, s_loc:s_loc + SUP]
                               .unsqueeze(2).broadcast_to([128, SUP, 64]))
                        nc.vector.tensor_tensor(
                            N4[:].rearrange("p (t f) -> p t f", f=64),
                            H4.rearrange("p (t f) -> p t f", f=64),
                            rex, mybir.AluOpType.mult)
                        if general_affine:
                            gex = (gb_sb[:, 0:64].unsqueeze(1)
                                   .broadcast_to([128, SUP, 64]))
                            bex = (gb_sb[:, 64:128].unsqueeze(1)
                                   .broadcast_to([128, SUP, 64]))
                            nc.vector.tensor_tensor(
                                N4[:].rearrange("p (t f) -> p t f", f=64),
                                N4[:].rearrange("p (t f) -> p t f", f=64),
                                gex, mybir.AluOpType.mult)
                            nc.vector.tensor_tensor(
                                N4[:].rearrange("p (t f) -> p t f", f=64),
                                N4[:].rearrange("p (t f) -> p t f", f=64),
                                bex, mybir.AluOpType.add)
                        H24 = h2_pool.tile([128, SUP * 64], MMDT)
                        nc.scalar.activation(H24[:], N4[:],
                                             mybir.ActivationFunctionType.Silu)

                        # shared PSUM tile: Q4 (cols 0:384) + paired h2t
                        # (cols 384:512, rows 0:128)
                        QT = psQT.tile([128, 384], F32)
                        for t in range(SUP):
                            h2t_pst = psT.tile([64, 128], MMDT)
                            nc.tensor.transpose(
                                h2t_pst[:],
                                H24[:, t * 64:(t + 1) * 64],
                                ident_sb[:])
                            h2t_sb = h2t_pool.tile([64, 128], MMDT)
                            if t % 8 == 0:
                                nc.vector.tensor_copy(h2t_sb[:], h2t_pst[:])
                            else:
                                nc.scalar.copy(h2t_sb[:].bitcast(F32),
                                               h2t_pst[:].bitcast(F32))
                            nc.tensor.matmul(
                                QT[:, t * 96:(t + 1) * 96],
                                h2t_sb[:],
                                b_sb[0:64, :],
                                start=True, stop=True)

                        deg4 = deg_bufs[(nt0 // SUP) % 2]
                        d3 = deg4[:].rearrange("p (t f) -> p t f", f=DEG_W)
                        q3 = QT[:, 0:SUP * 96].rearrange("p (t f) -> p t f", f=96)
                        a0ex = a3[:, :, 0:1].broadcast_to([128, SUP, 64])
                        nc.vector.scalar_tensor_tensor(
                            d3[:, :, 0:64], q3[:, :, 0:64], 0.0, a0ex,
                            mybir.AluOpType.bypass, mybir.AluOpType.mult)
                        for m_ in range(3):
                            a1ex = (a3[:, :, 1 + m_:2 + m_]
                                    .broadcast_to([128, SUP, 32]))
                            nc.vector.scalar_tensor_tensor(
                                d3[:, :, 64 + 32 * m_:96 + 32 * m_],
                                q3[:, :, 64:96], 0.0, a1ex,
                                mybir.AluOpType.bypass, mybir.AluOpType.mult)

                        for t in range(SUP):
                            nt = nt0 + t
                            w = winof(nt)
                            tin = nt - w * TPW
                            oh_eng = (nc.gpsimd
                                      if CONFIG["onehot_engine"] == "gpsimd"
                                      else nc.vector)
                            oh = oh_pool.tile([128, WIN], SCDT)
                            oh_eng.tensor_scalar(oh[:], iota_sb[:],
                                                 a3[:, t, 4:5], None,
                                                 mybir.AluOpType.is_equal)
                            if tin == 0:
                                if acc is not None:
                                    fl = fl_pool.tile([128, D_EMB], F32)
                                    nc.vector.tensor_copy(fl[:], acc[:, 0:D_EMB])
                                    nc.sync.dma_start(
                                        out_d[acc_win * 128:(acc_win + 1) * 128, :],
                                        fl[:])
                                acc = psA.tile([128, DEG_W], F32)
                                acc_win = w
                            is_last = (nt == NT - 1) or (winof(nt + 1) != w)
                            nc.tensor.matmul(
                                acc[:],
                                oh[:],
                                deg4[:, t * DEG_W:(t + 1) * DEG_W],
                                start=(tin == 0), stop=is_last,
                                skip_group_check=True)

            fl = fl_pool.tile([128, D_EMB], F32)
            nc.vector.tensor_copy(fl[:], acc[:, 0:D_EMB])
            nc.sync.dma_start(out_d[acc_win * 128:(acc_win + 1) * 128, :], fl[:])

    nc.finalize()
    return nc


def kernel(dst_input, src_attr, scalars, lin_w, lin_b, rad_w1, rad_g, rad_beta,
           rad_w2, rad_off, proj_w0, proj_b0, proj_w1, dst_index):
    dst_input = np.asarray(dst_input)
    src_attr = np.asarray(src_attr, np.float32)
    scalars = np.asarray(scalars, np.float32)
    lin_w = np.asarray(lin_w, np.float64)
    lin_b = np.asarray(lin_b, np.float64)
    rad_w1 = np.asarray(rad_w1, np.float32)
    rad_g = np.asarray(rad_g, np.float32)
    rad_beta = np.asarray(rad_beta, np.float32)
    rad_w2 = np.asarray(rad_w2, np.float64)
    rad_off = np.asarray(rad_off, np.float64)
    proj_w0 = np.asarray(proj_w0, np.float64)
    proj_b0 = np.asarray(proj_b0, np.float64)
    proj_w1 = np.asarray(proj_w1, np.float64)
    dst_index = np.asarray(dst_index)

    N = dst_input.shape[0]
    E = scalars.shape[0]
    out_dtype = dst_input.dtype

    # ---- host folds ----
    s0 = lin_w + lin_b                                   # [64]
    k0 = 1.0 / (math.sqrt(MUL0 + MUL1) * math.sqrt(AVG_AGG))
    k1 = 1.0 / (math.sqrt(MUL0 + 2 * MUL1) * math.sqrt(AVG_AGG))
    A0 = s0[:, None] * proj_w0[:MUL0, :]                 # [64, 64]
    A1 = s0[:, None] * proj_w1[:MUL0, :]                 # [64, 32]
    B = np.concatenate([rad_w2[:, 0:64] @ A0 * k0,
                        rad_w2[:, 64:128] @ A1 * k1], axis=1)  # [64, 96]
    c0 = rad_off[0:64] @ A0 * k0                         # [64]
    c1 = rad_off[64:128] @ A1 * k1                       # [32]
    W1c = rad_w1 - rad_w1.mean(axis=1, keepdims=True)    # centered: h-mu fold

    general_affine = not (np.allclose(rad_g, 1.0) and np.allclose(rad_beta, 0.0))

    # ---- edge sort and sharding ----
    NPC = (N + N_CORES - 1) // N_CORES                   # nodes per core
    NW = (NPC + WIN - 1) // WIN                          # windows per core
    order = np.argsort(dst_index, kind="stable")
    dst_sorted = dst_index[order]
    # boundaries of each (core, window) bucket; core k owns [k*NPC,(k+1)*NPC)
    # and its windows are 128-node ranges within that (last window clipped)
    bounds = [min(k * NPC + w * WIN, N)
              for k in range(N_CORES) for w in range(NW)]
    bounds.append(N)
    bucket_edges = np.searchsorted(dst_sorted, np.asarray(bounds))
    counts = np.diff(bucket_edges).reshape(N_CORES, NW)
    TPW = max(1, int(np.ceil(counts.max() / 128)))
    NT = NW * TPW
    NT = ((NT + SG * GRP - 1) // (SG * GRP)) * (SG * GRP)  # pad to super-group
    C = NT * 128

    key = (C, NW, TPW, NT, general_affine, tuple(sorted(CONFIG.items())))
    if key not in _PROGRAM_CACHE:
        _PROGRAM_CACHE[key] = build_program(C, NW, TPW, NT, general_affine)
    nc = _PROGRAM_CACHE[key]

    # ---- per-core input arrays ----
    iota = np.broadcast_to(np.arange(WIN, dtype=np.float32)[None, :],
                           (128, WIN)).copy()
    ident = np.eye(128, dtype=np.float32)
    w1c_f = W1c.astype(np.float32)
    b_f = np.ascontiguousarray(np.concatenate([B, B], axis=0).astype(np.float32))
    gbc = np.zeros((128, 128), np.float32)
    gbc[0, 0:64] = rad_g
    gbc[1, 0:64] = rad_beta

    in_maps = []
    for k in range(N_CORES):
        xt = np.zeros((C, 64), np.float32)
        aux = np.zeros((NT, 128, 5), np.float32)
        aux[:, :, 4] = -1.0
        for w in range(NW):
            lo, hi = bucket_edges[k * NW + w], bucket_edges[k * NW + w + 1]
            cnt = hi - lo
            if cnt == 0:
                continue
            eidx = order[lo:hi]
            base = w * TPW * 128
            xt[base:base + cnt] = scalars[eidx]
            a = aux.reshape(NT * 128, 5)
            a[base:base + cnt, 0] = src_attr[eidx, 0]
            a[base:base + cnt, 1:4] = src_attr[eidx, 1:4]
            a[base:base + cnt, 4] = (dst_sorted[lo:hi]
                                     - (k * NPC + w * WIN)).astype(np.float32)
        SGT_ = SG * GRP
        auxp = np.ascontiguousarray(
            aux.reshape(NT // SGT_, SGT_, 128, 5).transpose(0, 2, 1, 3)
            .reshape(NT // SGT_, 128, SGT_ * 5))
        m = {
            "xt": np.ascontiguousarray(xt.T),
            "aux": auxp,
            "w1c": w1c_f,
            "bmat": b_f,
            "iota": iota,
            "ident": ident,
        }
        if general_affine:
            m["gbc"] = gbc
        in_maps.append(m)

    global _LAST_IN_MAPS
    _LAST_IN_MAPS = in_maps
    res = run_bass_kernel_spmd(nc, in_maps, core_ids=list(range(N_CORES)))

    # ---- host assembly ----
    out = np.zeros((N, D_EMB), np.float64)
    for k in range(N_CORES):
        rows = res.results[k]["out"]                     # [NW*128, 160]
        lo = k * NPC
        hi = min(N, (k + 1) * NPC)
        out[lo:hi] = rows[0:hi - lo]
    # device o1 layout is m-major (64 + 32*m + v); reference is 64 + 3*v + m
    blk = out[:, 64:160].reshape(N, 3, 32)
    out[:, 64:160] = blk.transpose(0, 2, 1).reshape(N, 96)

    # host-side exact corrections (rad_off and proj_b0 terms)
    if np.any(proj_b0 != 0) or np.any(c0 != 0) or np.any(c1 != 0):
        cnt = np.bincount(dst_index, minlength=N).astype(np.float64)
        suma0 = np.bincount(dst_index, weights=src_attr[:, 0].astype(np.float64),
                            minlength=N)
        out[:, 0:64] += cnt[:, None] * (proj_b0 / math.sqrt(AVG_AGG))[None, :]
        out[:, 0:64] += suma0[:, None] * c0[None, :]
        for m_ in range(3):
            sa = np.bincount(dst_index,
                             weights=src_attr[:, 1 + m_].astype(np.float64),
                             minlength=N)
            out[:, 64 + m_::3][:, 0:32] += sa[:, None] * c1[None, :]

    return out.astype(out_dtype)



# revision 14
# speedup vs baseline: 1.4308x; 1.4308x over previous
"""Trainium2 Bass kernel for nn_DegreeEmbeddingNetwork (gnn_message_passing).

Strategy (8 NeuronCores, SPMD single program):
  - The reference collapses massively: node features are a constant broadcast
    (s0 = lin_w + lin_b) and the l=1 node block is structurally zero, so the
    whole per-edge computation is
        h   = scalars @ W1c                     (radial MLP layer 1, centered)
        h2  = silu(h * rstd)                    (LN folds to pure rms-norm)
        q0  = h2 @ B0 ; q1 = h2 @ B1            (B folds rad_w2 x TP x proj)
        deg = [a0*q0 | a1 (x) q1]               (160 wide)
        out = scatter_add(deg by dst) / sqrt(32)
  - Edges sorted by destination node on host; core k owns nodes
    [k*NPC,(k+1)*NPC), scatter-adds locally via one-hot matmuls into 128-node
    windows. One-hot tiles are precomputed on the host and DMAed (bf16).
  - All matmuls in bf16 (4x PE throughput vs fp32); H2 transposes are packed
    two tiles per PE transpose; rstd uses a magic-constant inverse-sqrt with
    two Newton steps on the DVE (keeps ACT pinned to one act-func table:
    Square/Silu/Copy live in the same set, so zero table reloads).
  - Elementwise work is spread across ACT/DVE/Pool; per-engine assignment of
    each op is a CONFIG knob.
"""

import contextlib
import math
import sys

sys.path.insert(0, "/opt/trn_rl_repo")

import numpy as np

import concourse.bacc as bacc
import concourse.tile as tile
from concourse import mybir
from concourse.bass_utils import run_bass_kernel_spmd

F32 = mybir.dt.float32
BF16 = mybir.dt.bfloat16
I32 = mybir.dt.int32

N_CORES = 8
MUL0, MUL1 = 64, 32
D_EMB = 160
RAD_HID = 64
AVG_AGG = 32.0
LN_EPS = 1e-5
WIN = 128          # nodes per scatter window
SGT = 16           # tiles per stats group (rsqrt batch)
SUP = 8            # tiles per elementwise batch (one PSUM bank of H)
MAGIC = 0x5F3759DF

# NOTE: gpsimd (Pool) cannot access PSUM, so Pool only gets SBUF-side work:
# the rsqrt chain and deg_v (fed from an SBUF copy of Q1).
CONFIG = {
    "norm_engine": "vector",
    "deg0_engine": "vector",
    "degv_engine": "gpsimd",    # 3x tensor_tensor w/ broadcast (stt illegal on Pool)
    "q1copy_engine": "scalar",
    "sq_engine": "scalar",
    "rsqrt_engine": "vector",
    "tcopy_engines": ("scalar", "scalar"),   # two [128,256] batched copies
    "flush_engine": "scalar",
    "chunk_tiles": 64,      # tiles per DMA chunk (multiple of SGT)
    "nr_iters": 1,
    "stage": 5,             # debug: truncate pipeline (5 = full)
}

_PROGRAM_CACHE = {}
_LAST_IN_MAPS = None


def _eng(nc, name):
    return {"vector": nc.vector, "scalar": nc.scalar, "gpsimd": nc.gpsimd}[name]


def build_program(NT, NW, win_of):
    """SPMD Bass program. NT tiles of 128 edges (window-major, padded), NW
    windows of 128 nodes, win_of[t] = window of tile t (non-decreasing)."""
    CH = CONFIG["chunk_tiles"]
    assert NT % SGT == 0 and CH % SGT == 0 and NT % CH == 0
    C = NT * 128
    n_chunks = NT // CH
    chunk_lo = [c * CH for c in range(n_chunks)]
    chunk_sz = [CH] * n_chunks

    nc = bacc.Bacc("TRN2", target_bir_lowering=False, debug=False,
                   num_devices=N_CORES)

    xt_d = nc.dram_tensor("xt", [64, C], BF16, kind="ExternalInput").ap()
    oh_d = nc.dram_tensor("oh", [128, C], BF16, kind="ExternalInput").ap()
    aux_d = nc.dram_tensor("aux", [128, NT * 4], F32, kind="ExternalInput").ap()
    w1_d = nc.dram_tensor("w1c", [64, 64], BF16, kind="ExternalInput").ap()
    b0_d = nc.dram_tensor("b0", [128, 128], BF16, kind="ExternalInput").ap()
    b1_d = nc.dram_tensor("b1", [128, 64], BF16, kind="ExternalInput").ap()
    id_d = nc.dram_tensor("ident", [128, 128], BF16, kind="ExternalInput").ap()
    out_d = nc.dram_tensor("out", [NW * 128, D_EMB], F32,
                           kind="ExternalOutput").ap()

    Sq = mybir.ActivationFunctionType.Square
    Silu = mybir.ActivationFunctionType.Silu
    Add = mybir.AluOpType.add
    Mult = mybir.AluOpType.mult
    Bypass = mybir.AluOpType.bypass

    with tile.TileContext(nc) as tc:
        with contextlib.ExitStack() as _es:
            _p = lambda *a, **k: _es.enter_context(tc.tile_pool(*a, **k))
            cpool = _p(name="consts", bufs=1)
            xtc_pool = _p(name="xtc", bufs=2)
            ohc_pool = _p(name="ohc", bufs=2)
            auxc_pool = _p(name="auxc", bufs=2)
            sq_pool = _p(name="sq", bufs=2)
            n8_pool = _p(name="n8", bufs=2)
            h2_pool = _p(name="h2", bufs=2)
            h2t_pool = _p(name="h2t", bufs=2)
            q1s_pool = _p(name="q1s", bufs=2)
            st_pool = _p(name="st", bufs=3)
            deg_pool = _p(name="deg", bufs=2)
            fl_pool = _p(name="fl", bufs=3)
            psH = _p(name="psH", bufs=3, space="PSUM")
            psQ0 = _p(name="psQ0", bufs=2, space="PSUM")
            psMIX = _p(name="psMIX", bufs=2, space="PSUM")
            psA = _p(name="psA", bufs=1, space="PSUM")
            w1_sb = cpool.tile([64, 64], BF16)
            nc.sync.dma_start(w1_sb[:], w1_d[:])
            b0_sb = cpool.tile([128, 128], BF16)
            nc.sync.dma_start(b0_sb[:], b0_d[:])
            b1_sb = cpool.tile([128, 64], BF16)
            nc.sync.dma_start(b1_sb[:], b1_d[:])
            id_sb = cpool.tile([128, 128], BF16)
            nc.sync.dma_start(id_sb[:], id_d[:])

            chunk_xt, chunk_oh, chunk_aux = {}, {}, {}

            def issue_chunk(c):
                lo, sz = chunk_lo[c], chunk_sz[c]
                xg = xtc_pool.tile([64, sz * 128], BF16, tag=f"xt{c % 2}")
                nc.sync.dma_start(xg[:], xt_d[:, lo * 128:(lo + sz) * 128])
                og = ohc_pool.tile([128, sz * 128], BF16, tag=f"oh{c % 2}")
                nc.sync.dma_start(og[:], oh_d[:, lo * 128:(lo + sz) * 128])
                ag = auxc_pool.tile([128, sz * 4], F32, tag=f"aux{c % 2}")
                nc.sync.dma_start(ag[:], aux_d[:, lo * 4:(lo + sz) * 4])
                chunk_xt[c], chunk_oh[c], chunk_aux[c] = xg, og, ag

            issue_chunk(0)
            if n_chunks > 1:
                issue_chunk(1)

            acc = None
            acc_win = -1

            for sg in range(NT // SGT):
                nt0 = sg * SGT
                cidx = nt0 // CH
                if nt0 == chunk_lo[cidx] and cidx >= 1 and cidx + 1 < n_chunks:
                    issue_chunk(cidx + 1)
                xtg, ohg, auxg = chunk_xt[cidx], chunk_oh[cidx], chunk_aux[cidx]
                lt = nt0 - chunk_lo[cidx]   # tile offset within chunk

                # ---- pass 1: MM1 + square + reduce ----
                ssq = st_pool.tile([128, SGT], F32, tag="ssq")
                Hs = []
                for su in range(2):
                    H = psH.tile([128, SUP * 64], F32)
                    Hs.append(H)
                    for t in range(SUP):
                        gt = lt + su * SUP + t
                        nc.tensor.matmul(
                            H[:, t * 64:(t + 1) * 64],
                            xtg[:, gt * 128:(gt + 1) * 128],
                            w1_sb[:], start=True, stop=True)
                    sq8 = sq_pool.tile([128, SUP * 64], F32)
                    _eng(nc, CONFIG["sq_engine"]).activation(sq8[:], H[:], Sq)
                    nc.vector.tensor_reduce(
                        ssq[:, su * SUP:(su + 1) * SUP],
                        sq8[:].rearrange("p (t f) -> p t f", f=64),
                        axis=mybir.AxisListType.X, op=Add)

                # ---- rstd = rsqrt(ssq/64 + eps): magic seed + Newton ----
                # seed bits = MAGIC - bits(v)/2, done in f32 value domain via
                # dtype-converting copies (int ALU scalars are blocked)
                rsq = _eng(nc, CONFIG["rsqrt_engine"])
                v = st_pool.tile([128, SGT], F32, tag="v")
                rsq.tensor_scalar(v[:], ssq[:], 1.0 / 64.0, LN_EPS,
                                  Mult, Add)
                u = st_pool.tile([128, SGT], F32, tag="u")
                rsq.tensor_copy(u[:], v[:].bitcast(I32))
                yf = st_pool.tile([128, SGT], F32, tag="yf")
                rsq.tensor_scalar(yf[:], u[:], -0.5, float(MAGIC),
                                  Mult, Add)
                y = st_pool.tile([128, SGT], F32, tag="y0")
                rsq.tensor_copy(y[:].bitcast(I32), yf[:])
                for it in range(CONFIG["nr_iters"]):
                    t2 = st_pool.tile([128, SGT], F32, tag=f"t2_{it}")
                    rsq.tensor_tensor(t2[:], y[:], y[:], Mult)
                    w_ = st_pool.tile([128, SGT], F32, tag=f"w_{it}")
                    rsq.scalar_tensor_tensor(w_[:], t2[:], -0.5, v[:],
                                             Mult, Mult)
                    y2 = st_pool.tile([128, SGT], F32, tag=f"y_{it + 1}")
                    rsq.scalar_tensor_tensor(y2[:], w_[:], 1.5, y[:],
                                             Add, Mult)
                    y = y2
                rstd = y

                # ---- pass 2 ----
                if CONFIG["stage"] < 2:
                    continue
                for su in range(2):
                    H = Hs[su]
                    N8 = n8_pool.tile([128, SUP * 64], F32)
                    rex = (rstd[:, su * SUP:(su + 1) * SUP]
                           .unsqueeze(2).broadcast_to([128, SUP, 64]))
                    _eng(nc, CONFIG["norm_engine"]).scalar_tensor_tensor(
                        N8[:].rearrange("p (t f) -> p t f", f=64),
                        H[:].rearrange("p (t f) -> p t f", f=64),
                        0.0, rex, Bypass, Mult)
                    H24 = h2_pool.tile([128, SUP * 64], BF16)
                    nc.scalar.activation(H24[:], N8[:], Silu)
                    if CONFIG["stage"] < 3:
                        continue

                    mix = psMIX.tile([128, 512], F32)
                    q1v = mix[:, 0:256]
                    Q0 = psQ0.tile([128, SUP * 64], F32)
                    h2t = h2t_pool.tile([128, 512], BF16)
                    for half in range(2):
                        for p in (2 * half, 2 * half + 1):
                            nc.tensor.transpose(
                                mix[:, 256 + p * 64:320 + p * 64].bitcast(BF16),
                                H24[:, p * 128:(p + 1) * 128], id_sb[:])
                        src_ap = mix[:, 256 + half * 128:384 + half * 128]
                        dst_ap = h2t[:, half * 256:(half + 1) * 256]
                        eng = CONFIG["tcopy_engines"][half % 2]
                        if eng == "scalar":
                            nc.scalar.copy(dst_ap, src_ap.bitcast(BF16))
                        else:
                            _eng(nc, eng).tensor_copy(dst_ap,
                                                      src_ap.bitcast(BF16))
                        for p in (2 * half, 2 * half + 1):
                            # full-128-partition lhsT; [B;0]/[0;B] rhs picks
                            # the tile half (partition-offset lhsT miscompiles)
                            lhs = h2t[:, p * 128:(p + 1) * 128]
                            for j in range(2):
                                tt = 2 * p + j
                                nc.tensor.matmul(
                                    Q0[:, tt * 64:(tt + 1) * 64], lhs,
                                    b0_sb[:, j * 64:(j + 1) * 64],
                                    start=True, stop=True)
                                nc.tensor.matmul(
                                    q1v[:, tt * 32:(tt + 1) * 32], lhs,
                                    b1_sb[:, j * 32:(j + 1) * 32],
                                    start=True, stop=True)

                    if CONFIG["stage"] < 4:
                        continue
                    q1s = q1s_pool.tile([128, SUP * 32], BF16)
                    qeng = CONFIG["q1copy_engine"]
                    if qeng == "scalar":
                        nc.scalar.copy(q1s[:], q1v)
                    else:
                        _eng(nc, qeng).tensor_copy(q1s[:], q1v)

                    a3 = (auxg[:, (lt + su * SUP) * 4:(lt + (su + 1) * SUP) * 4]
                          .rearrange("p (t f) -> p t f", f=4))
                    deg = deg_pool.tile([128, SUP * D_EMB], BF16)
                    d3 = deg[:].rearrange("p (t f) -> p t f", f=D_EMB)
                    a0ex = a3[:, :, 0:1].broadcast_to([128, SUP, 64])
                    _eng(nc, CONFIG["deg0_engine"]).scalar_tensor_tensor(
                        d3[:, :, 0:64],
                        Q0[:].rearrange("p (t f) -> p t f", f=64),
                        0.0, a0ex, Bypass, Mult)
                    q1ex = q1s[:].rearrange("p (t v) -> p t v", v=32)
                    for m_ in range(3):
                        a1ex = (a3[:, :, 1 + m_:2 + m_]
                                .broadcast_to([128, SUP, 32]))
                        deng = CONFIG["degv_engine"]
                        if deng == "gpsimd":
                            nc.gpsimd.tensor_tensor(
                                d3[:, :, 64 + 32 * m_:96 + 32 * m_],
                                q1ex, a1ex, Mult)
                        else:
                            _eng(nc, deng).scalar_tensor_tensor(
                                d3[:, :, 64 + 32 * m_:96 + 32 * m_],
                                q1ex, 0.0, a1ex, Bypass, Mult)

                    if CONFIG["stage"] < 5:
                        continue
                    for t in range(SUP):
                        nt = nt0 + su * SUP + t
                        w = win_of[nt]
                        if w != acc_win:
                            if acc is not None:
                                fl = fl_pool.tile([128, D_EMB], F32)
                                feng = CONFIG["flush_engine"]
                                if feng == "scalar":
                                    nc.scalar.copy(fl[:], acc[:])
                                else:
                                    _eng(nc, feng).tensor_copy(fl[:], acc[:])
                                nc.sync.dma_start(
                                    out_d[acc_win * 128:(acc_win + 1) * 128, :],
                                    fl[:])
                            acc = psA.tile([128, D_EMB], F32)
                            acc_win = w
                        is_last = (nt == NT - 1) or (win_of[nt + 1] != w)
                        gt = lt + su * SUP + t
                        nc.tensor.matmul(
                            acc[:],
                            ohg[:, gt * 128:(gt + 1) * 128],
                            deg[:, t * D_EMB:(t + 1) * D_EMB],
                            start=(w != win_of[nt - 1] if nt > 0 else True),
                            stop=is_last, skip_group_check=True)

            if acc is None:
                acc = psA.tile([128, D_EMB], F32)
                nc.vector.memset(acc[:].bitcast(F32), 0.0)
                acc_win = 0
            fl = fl_pool.tile([128, D_EMB], F32)
            if CONFIG["flush_engine"] == "scalar":
                nc.scalar.copy(fl[:], acc[:])
            else:
                _eng(nc, CONFIG["flush_engine"]).tensor_copy(fl[:], acc[:])
            nc.sync.dma_start(out_d[acc_win * 128:(acc_win + 1) * 128, :], fl[:])

    nc.finalize()
    return nc


def kernel(dst_input, src_attr, scalars, lin_w, lin_b, rad_w1, rad_g, rad_beta,
           rad_w2, rad_off, proj_w0, proj_b0, proj_w1, dst_index):
    dst_input = np.asarray(dst_input)
    src_attr = np.asarray(src_attr, np.float32)
    scalars = np.asarray(scalars, np.float32)
    lin_w = np.asarray(lin_w, np.float64)
    lin_b = np.asarray(lin_b, np.float64)
    rad_w1 = np.asarray(rad_w1, np.float64)
    rad_g = np.asarray(rad_g, np.float32)
    rad_beta = np.asarray(rad_beta, np.float32)
    rad_w2 = np.asarray(rad_w2, np.float64)
    rad_off = np.asarray(rad_off, np.float64)
    proj_w0 = np.asarray(proj_w0, np.float64)
    proj_b0 = np.asarray(proj_b0, np.float64)
    proj_w1 = np.asarray(proj_w1, np.float64)
    dst_index = np.asarray(dst_index)

    N = dst_input.shape[0]
    E = scalars.shape[0]
    out_dtype = dst_input.dtype
    bf16 = mybir.dt.np(BF16)

    assert np.allclose(rad_g, 1.0) and np.allclose(rad_beta, 0.0), \
        "general affine LN not supported in this build"

    # ---- host folds ----
    s0 = lin_w + lin_b                                   # [64]
    k0 = 1.0 / (math.sqrt(MUL0 + MUL1) * math.sqrt(AVG_AGG))
    k1 = 1.0 / (math.sqrt(MUL0 + 2 * MUL1) * math.sqrt(AVG_AGG))
    A0 = s0[:, None] * proj_w0[:MUL0, :]                 # [64, 64]
    A1 = s0[:, None] * proj_w1[:MUL0, :]                 # [64, 32]
    B0 = rad_w2[:, 0:64] @ A0 * k0                       # [64, 64]
    B1 = rad_w2[:, 64:128] @ A1 * k1                     # [64, 32]
    c0 = rad_off[0:64] @ A0 * k0                         # [64]
    c1 = rad_off[64:128] @ A1 * k1                       # [32]
    W1c = rad_w1 - rad_w1.mean(axis=1, keepdims=True)    # centered: h-mu fold

    # ---- edge sort and sharding ----
    NPC = (N + N_CORES - 1) // N_CORES                   # nodes per core
    NW = (NPC + WIN - 1) // WIN                          # windows per core
    order = np.argsort(dst_index, kind="stable")
    dst_sorted = dst_index[order]
    bounds = [min(k * NPC + w * WIN, N)
              for k in range(N_CORES) for w in range(NW)]
    bounds.append(N)
    bucket_edges = np.searchsorted(dst_sorted, np.asarray(bounds))
    counts = np.diff(bucket_edges).reshape(N_CORES, NW)
    # per-window tile count: max over cores (SPMD program is shared)
    tpw = np.maximum(1, -(-counts.max(axis=0) // 128))   # [NW]
    NT = int(tpw.sum())
    NT = ((NT + CONFIG["chunk_tiles"] - 1)
          // CONFIG["chunk_tiles"]) * CONFIG["chunk_tiles"]
    tile_off = np.concatenate([[0], np.cumsum(tpw)]).astype(int)
    win_of = []
    for w in range(NW):
        win_of += [w] * int(tpw[w])
    win_of += [NW - 1] * (NT - len(win_of))              # pad tiles
    win_of = tuple(win_of)
    C = NT * 128

    key = (NT, NW, win_of, tuple(sorted((k, str(v)) for k, v in CONFIG.items())))
    if key not in _PROGRAM_CACHE:
        _PROGRAM_CACHE[key] = build_program(NT, NW, list(win_of))
    nc = _PROGRAM_CACHE[key]

    # ---- per-core input arrays ----
    w1_bf = W1c.astype(np.float32).astype(bf16)
    z64 = np.zeros((64, 64)); z32 = np.zeros((64, 32))
    b0_bf = np.ascontiguousarray(np.hstack([np.vstack([B0, z64]),
                                            np.vstack([z64, B0])])
                                 .astype(np.float32)).astype(bf16)
    b1_bf = np.ascontiguousarray(np.hstack([np.vstack([B1, z32]),
                                            np.vstack([z32, B1])])
                                 .astype(np.float32)).astype(bf16)
    ident = np.eye(128, dtype=np.float32).astype(bf16)

    in_maps = []
    for k in range(N_CORES):
        xt = np.zeros((NT * 128, 64), bf16)
        oh = np.zeros((NT * 128, 128), bf16)
        aux = np.zeros((NT * 128, 4), np.float32)
        for w in range(NW):
            lo, hi = bucket_edges[k * NW + w], bucket_edges[k * NW + w + 1]
            cnt = hi - lo
            if cnt == 0:
                continue
            eidx = order[lo:hi]
            base = int(tile_off[w]) * 128
            rows = base + np.arange(cnt)
            xt[rows] = scalars[eidx].astype(bf16)
            offs = (dst_sorted[lo:hi] - (k * NPC + w * WIN)).astype(int)
            oh[rows, offs] = np.float32(1.0)
            aux[rows] = src_attr[eidx]
        m = {
            "xt": np.ascontiguousarray(xt.T),
            "oh": np.ascontiguousarray(
                oh.reshape(NT, 128, 128).transpose(1, 0, 2)
                .reshape(128, NT * 128)),
            "aux": np.ascontiguousarray(
                aux.reshape(NT, 128, 4).transpose(1, 0, 2)
                .reshape(128, NT * 4)),
            "w1c": w1_bf,
            "b0": b0_bf,
            "b1": b1_bf,
            "ident": ident,
        }
        in_maps.append(m)

    global _LAST_IN_MAPS
    _LAST_IN_MAPS = in_maps
    res = run_bass_kernel_spmd(nc, in_maps, core_ids=list(range(N_CORES)))

    # ---- host assembly ----
    out = np.zeros((N, D_EMB), np.float64)
    for k in range(N_CORES):
        rows = res.results[k]["out"]                     # [NW*128, 160]
        lo = k * NPC
        hi = min(N, (k + 1) * NPC)
        out[lo:hi] = rows[0:hi - lo]
    # device o1 layout is m-major (64 + 32*m + v); reference is 64 + 3*v + m
    blk = out[:, 64:160].reshape(N, 3, 32)
    out[:, 64:160] = blk.transpose(0, 2, 1).reshape(N, 96)

    # host-side exact corrections (rad_off and proj_b0 terms)
    if np.any(proj_b0 != 0) or np.any(c0 != 0) or np.any(c1 != 0):
        cnt = np.bincount(dst_index, minlength=N).astype(np.float64)
        suma0 = np.bincount(dst_index, weights=src_attr[:, 0].astype(np.float64),
                            minlength=N)
        out[:, 0:64] += cnt[:, None] * (proj_b0 / math.sqrt(AVG_AGG))[None, :]
        out[:, 0:64] += suma0[:, None] * c0[None, :]
        for m_ in range(3):
            sa = np.bincount(dst_index,
                             weights=src_attr[:, 1 + m_].astype(np.float64),
                             minlength=N)
            out[:, 64 + m_::3][:, 0:32] += sa[:, None] * c1[None, :]

    return out.astype(out_dtype)


# revision 17
# speedup vs baseline: 1.7114x; 1.1961x over previous
"""Trainium2 Bass kernel for nn_DegreeEmbeddingNetwork (gnn_message_passing).

Strategy (8 NeuronCores, SPMD single program):
  - The reference collapses massively: node features are a constant broadcast
    (s0 = lin_w + lin_b) and the l=1 node block is structurally zero, so the
    whole per-edge computation is
        h   = scalars @ W1c                     (radial MLP layer 1, centered)
        h2  = silu(h * rstd)                    (LN folds to pure rms-norm)
        q0  = h2 @ B0 ; q1 = h2 @ B1            (B folds rad_w2 x TP x proj)
        deg = [a0*q0 | a1 (x) q1]               (160 wide)
        out = scatter_add(deg by dst) / sqrt(32)
  - Edges sorted by destination node on host; core k owns nodes
    [k*NPC,(k+1)*NPC), scatter-adds locally via one-hot matmuls into 128-node
    windows. One-hot tiles are precomputed on the host and DMAed (bf16).
  - All matmuls in bf16 (4x PE throughput vs fp32); H2 transposes are packed
    two tiles per PE transpose; rstd uses a magic-constant inverse-sqrt with
    two Newton steps on the DVE (keeps ACT pinned to one act-func table:
    Square/Silu/Copy live in the same set, so zero table reloads).
  - Elementwise work is spread across ACT/DVE/Pool; per-engine assignment of
    each op is a CONFIG knob.
"""

import contextlib
import math
import sys

sys.path.insert(0, "/opt/trn_rl_repo")

import numpy as np

import concourse.bacc as bacc
import concourse.tile as tile
from concourse import mybir
from concourse.bass_utils import run_bass_kernel_spmd

F32 = mybir.dt.float32
BF16 = mybir.dt.bfloat16
I32 = mybir.dt.int32

N_CORES = 8
MUL0, MUL1 = 64, 32
D_EMB = 160
RAD_HID = 64
AVG_AGG = 32.0
LN_EPS = 1e-5
WIN = 128          # nodes per scatter window
SGT = 16           # tiles per stats group (rsqrt batch)
SUP = 8            # tiles per elementwise batch (one PSUM bank of H)
MAGIC = 0x5F3759DF

# NOTE: gpsimd (Pool) cannot access PSUM, so Pool only gets SBUF-side work:
# the rsqrt chain and deg_v (fed from an SBUF copy of Q1).
CONFIG = {
    "norm_engine": "vector",
    "deg0_engine": "vector",
    "degv_engine": "gpsimd",    # 3x tensor_tensor w/ broadcast (stt illegal on Pool)
    "q1copy_engine": "scalar",
    "sq_engine": "scalar",
    "rsqrt_engine": "vector",
    "tcopy_engines": ("scalar", "scalar"),   # two [128,256] batched copies
    "flush_engine": "scalar",
    "chunk_tiles": 32,      # tiles per DMA chunk (multiple of SGT)
    "nr_iters": 1,
    "stage": 5,             # debug: truncate pipeline (5 = full)
}

_PROGRAM_CACHE = {}
_LAST_IN_MAPS = None


def _eng(nc, name):
    return {"vector": nc.vector, "scalar": nc.scalar, "gpsimd": nc.gpsimd}[name]


def build_program(NT, NW, win_of):
    """SPMD Bass program. NT tiles of 128 edges (window-major, padded), NW
    windows of 128 nodes, win_of[t] = window of tile t (non-decreasing)."""
    CH = CONFIG["chunk_tiles"]
    assert NT % SGT == 0 and CH % SGT == 0 and NT % CH == 0
    C = NT * 128
    n_chunks = NT // CH
    chunk_lo = [c * CH for c in range(n_chunks)]
    chunk_sz = [CH] * n_chunks

    nc = bacc.Bacc("TRN2", target_bir_lowering=False, debug=False,
                   num_devices=N_CORES)

    xt_d = nc.dram_tensor("xt", [64, C], BF16, kind="ExternalInput").ap()
    oh_d = nc.dram_tensor("oh", [128, C], BF16, kind="ExternalInput").ap()
    aux_d = nc.dram_tensor("aux", [128, NT * 4], F32, kind="ExternalInput").ap()
    w1_d = nc.dram_tensor("w1c", [64, 64], BF16, kind="ExternalInput").ap()
    b0_d = nc.dram_tensor("b0", [128, 128], BF16, kind="ExternalInput").ap()
    b1_d = nc.dram_tensor("b1", [128, 64], BF16, kind="ExternalInput").ap()
    id_d = nc.dram_tensor("ident", [128, 128], BF16, kind="ExternalInput").ap()
    out_d = nc.dram_tensor("out", [NW * 128, D_EMB], F32,
                           kind="ExternalOutput").ap()

    Sq = mybir.ActivationFunctionType.Square
    Silu = mybir.ActivationFunctionType.Silu
    Add = mybir.AluOpType.add
    Mult = mybir.AluOpType.mult
    Bypass = mybir.AluOpType.bypass

    with tile.TileContext(nc) as tc:
        with contextlib.ExitStack() as _es:
            _p = lambda *a, **k: _es.enter_context(tc.tile_pool(*a, **k))
            cpool = _p(name="consts", bufs=1)
            xtc_pool = _p(name="xtc", bufs=2)
            ohc_pool = _p(name="ohc", bufs=2)
            auxc_pool = _p(name="auxc", bufs=2)
            sq_pool = _p(name="sq", bufs=2)
            n8_pool = _p(name="n8", bufs=2)
            h2_pool = _p(name="h2", bufs=2)
            h2t_pool = _p(name="h2t", bufs=2)
            q1s_pool = _p(name="q1s", bufs=2)
            st_pool = _p(name="st", bufs=3)
            deg_pool = _p(name="deg", bufs=2)
            fl_pool = _p(name="fl", bufs=3)
            psH = _p(name="psH", bufs=3, space="PSUM")
            psQ0 = _p(name="psQ0", bufs=2, space="PSUM")
            psMIX = _p(name="psMIX", bufs=2, space="PSUM")
            psA = _p(name="psA", bufs=1, space="PSUM")
            w1_sb = cpool.tile([64, 64], BF16)
            nc.sync.dma_start(w1_sb[:], w1_d[:])
            b0_sb = cpool.tile([128, 128], BF16)
            nc.sync.dma_start(b0_sb[:], b0_d[:])
            b1_sb = cpool.tile([128, 64], BF16)
            nc.sync.dma_start(b1_sb[:], b1_d[:])
            id_sb = cpool.tile([128, 128], BF16)
            nc.sync.dma_start(id_sb[:], id_d[:])

            chunk_xt, chunk_oh, chunk_aux = {}, {}, {}

            def issue_chunk(c):
                lo, sz = chunk_lo[c], chunk_sz[c]
                xg = xtc_pool.tile([64, sz * 128], BF16, tag=f"xt{c % 2}")
                nc.sync.dma_start(xg[:], xt_d[:, lo * 128:(lo + sz) * 128])
                og = ohc_pool.tile([128, sz * 128], BF16, tag=f"oh{c % 2}")
                nc.sync.dma_start(og[:], oh_d[:, lo * 128:(lo + sz) * 128])
                ag = auxc_pool.tile([128, sz * 4], F32, tag=f"aux{c % 2}")
                nc.sync.dma_start(ag[:], aux_d[:, lo * 4:(lo + sz) * 4])
                chunk_xt[c], chunk_oh[c], chunk_aux[c] = xg, og, ag

            issue_chunk(0)
            if n_chunks > 1:
                issue_chunk(1)

            acc = None
            acc_win = -1
            state = {}

            def emit_pass1(sg):
                nt0 = sg * SGT
                cidx = nt0 // CH
                if nt0 == chunk_lo[cidx] and cidx >= 1 and cidx + 1 < n_chunks:
                    issue_chunk(cidx + 1)
                xtg = chunk_xt[cidx]
                lt = nt0 - chunk_lo[cidx]
                if sg not in state:
                    state[sg] = {"ssq": st_pool.tile([128, SGT], F32, name="ssq",
                                                     tag="ssq"), "H": {}}
                ssq = state[sg]["ssq"]
                for su in range(2):
                    H = psH.tile([128, SUP * 64], F32, tag="H")
                    state[sg]["H"][su] = H
                    for t in range(SUP):
                        gt = lt + su * SUP + t
                        nc.tensor.matmul(
                            H[:, t * 64:(t + 1) * 64],
                            xtg[:, gt * 128:(gt + 1) * 128],
                            w1_sb[:], start=True, stop=True)
                    sq8 = sq_pool.tile([128, SUP * 64], F32, tag="sq8")
                    _eng(nc, CONFIG["sq_engine"]).activation(sq8[:], H[:], Sq)
                    nc.vector.tensor_reduce(
                        ssq[:, su * SUP:(su + 1) * SUP],
                        sq8[:].rearrange("p (t f) -> p t f", f=64),
                        axis=mybir.AxisListType.X, op=Add)

            def emit_rsqrt(sg):
                ssq = state[sg]["ssq"]
                rsq = _eng(nc, CONFIG["rsqrt_engine"])
                v = st_pool.tile([128, SGT], F32, tag="v")
                rsq.tensor_scalar(v[:], ssq[:], 1.0 / 64.0, LN_EPS, Mult, Add)
                u = st_pool.tile([128, SGT], F32, tag="u")
                rsq.tensor_copy(u[:], v[:].bitcast(I32))
                yf = st_pool.tile([128, SGT], F32, tag="yf")
                rsq.tensor_scalar(yf[:], u[:], -0.5, float(MAGIC), Mult, Add)
                y = st_pool.tile([128, SGT], F32, tag="y0")
                rsq.tensor_copy(y[:].bitcast(I32), yf[:])
                for it in range(CONFIG["nr_iters"]):
                    t2 = st_pool.tile([128, SGT], F32, tag=f"t2_{it}")
                    rsq.tensor_tensor(t2[:], y[:], y[:], Mult)
                    w_ = st_pool.tile([128, SGT], F32, tag=f"w_{it}")
                    rsq.scalar_tensor_tensor(w_[:], t2[:], -0.5, v[:],
                                             Mult, Mult)
                    y2 = st_pool.tile([128, SGT], F32, tag=f"y_{it + 1}")
                    rsq.scalar_tensor_tensor(y2[:], w_[:], 1.5, y[:],
                                             Add, Mult)
                    y = y2
                state[sg]["rstd"] = y

            def emit_pass2su(sg, su):
                nonlocal acc, acc_win
                nt0 = sg * SGT
                cidx = nt0 // CH
                ohg, auxg = chunk_oh[cidx], chunk_aux[cidx]
                lt = nt0 - chunk_lo[cidx]
                H = state[sg]["H"][su]
                rstd = state[sg]["rstd"]
                N8 = n8_pool.tile([128, SUP * 64], F32, tag="N8")
                rex = (rstd[:, su * SUP:(su + 1) * SUP]
                       .unsqueeze(2).broadcast_to([128, SUP, 64]))
                _eng(nc, CONFIG["norm_engine"]).scalar_tensor_tensor(
                    N8[:].rearrange("p (t f) -> p t f", f=64),
                    H[:].rearrange("p (t f) -> p t f", f=64),
                    0.0, rex, Bypass, Mult)
                H24 = h2_pool.tile([128, SUP * 64], BF16, tag="H24")
                nc.scalar.activation(H24[:], N8[:], Silu)

                mix = psMIX.tile([128, 512], F32, tag="mix")
                q1v = mix[:, 0:256]
                Q0 = psQ0.tile([128, SUP * 64], F32, tag="Q0")
                h2t = h2t_pool.tile([128, 512], BF16, tag="h2t")
                for half in range(2):
                    for p in (2 * half, 2 * half + 1):
                        nc.tensor.transpose(
                            mix[:, 256 + p * 64:320 + p * 64].bitcast(BF16),
                            H24[:, p * 128:(p + 1) * 128], id_sb[:])
                    src_ap = mix[:, 256 + half * 128:384 + half * 128]
                    dst_ap = h2t[:, half * 256:(half + 1) * 256]
                    eng = CONFIG["tcopy_engines"][half % 2]
                    if eng == "scalar":
                        nc.scalar.copy(dst_ap, src_ap.bitcast(BF16))
                    else:
                        _eng(nc, eng).tensor_copy(dst_ap, src_ap.bitcast(BF16))
                    for p in (2 * half, 2 * half + 1):
                        lhs = h2t[:, p * 128:(p + 1) * 128]
                        for j in range(2):
                            tt = 2 * p + j
                            nc.tensor.matmul(
                                Q0[:, tt * 64:(tt + 1) * 64], lhs,
                                b0_sb[:, j * 64:(j + 1) * 64],
                                start=True, stop=True)
                            nc.tensor.matmul(
                                q1v[:, tt * 32:(tt + 1) * 32], lhs,
                                b1_sb[:, j * 32:(j + 1) * 32],
                                start=True, stop=True)

                q1s = q1s_pool.tile([128, SUP * 32], BF16, tag="q1s")
                qeng = CONFIG["q1copy_engine"]
                if qeng == "scalar":
                    nc.scalar.copy(q1s[:], q1v)
                else:
                    _eng(nc, qeng).tensor_copy(q1s[:], q1v)

                a3 = (auxg[:, (lt + su * SUP) * 4:(lt + (su + 1) * SUP) * 4]
                      .rearrange("p (t f) -> p t f", f=4))
                deg = deg_pool.tile([128, SUP * D_EMB], BF16, tag="deg")
                d3 = deg[:].rearrange("p (t f) -> p t f", f=D_EMB)
                a0ex = a3[:, :, 0:1].broadcast_to([128, SUP, 64])
                _eng(nc, CONFIG["deg0_engine"]).scalar_tensor_tensor(
                    d3[:, :, 0:64],
                    Q0[:].rearrange("p (t f) -> p t f", f=64),
                    0.0, a0ex, Bypass, Mult)
                q1ex = q1s[:].rearrange("p (t v) -> p t v", v=32)
                for m_ in range(3):
                    a1ex = (a3[:, :, 1 + m_:2 + m_]
                            .broadcast_to([128, SUP, 32]))
                    deng = CONFIG["degv_engine"]
                    if deng == "gpsimd":
                        nc.gpsimd.tensor_tensor(
                            d3[:, :, 64 + 32 * m_:96 + 32 * m_],
                            q1ex, a1ex, Mult)
                    else:
                        _eng(nc, deng).scalar_tensor_tensor(
                            d3[:, :, 64 + 32 * m_:96 + 32 * m_],
                            q1ex, 0.0, a1ex, Bypass, Mult)

                for t in range(SUP):
                    nt = nt0 + su * SUP + t
                    w = win_of[nt]
                    if w != acc_win:
                        if acc is not None:
                            fl = fl_pool.tile([128, D_EMB], F32, tag="fl")
                            feng = CONFIG["flush_engine"]
                            if feng == "scalar":
                                nc.scalar.copy(fl[:], acc[:])
                            else:
                                _eng(nc, feng).tensor_copy(fl[:], acc[:])
                            nc.sync.dma_start(
                                out_d[acc_win * 128:(acc_win + 1) * 128, :],
                                fl[:])
                        acc = psA.tile([128, D_EMB], F32, tag="acc")
                        acc_win = w
                    is_last = (nt == NT - 1) or (win_of[nt + 1] != w)
                    gt = lt + su * SUP + t
                    nc.tensor.matmul(
                        acc[:],
                        ohg[:, gt * 128:(gt + 1) * 128],
                        deg[:, t * D_EMB:(t + 1) * D_EMB],
                        start=(w != win_of[nt - 1] if nt > 0 else True),
                        stop=is_last, skip_group_check=True)

            # software pipeline: pass1(sg+1) interleaves with pass2(sg)
            NSG = NT // SGT
            emit_pass1(0)
            emit_rsqrt(0)
            for sg in range(NSG):
                if sg + 1 < NSG:
                    emit_pass1(sg + 1)
                emit_pass2su(sg, 0)
                if sg + 1 < NSG:
                    emit_rsqrt(sg + 1)
                emit_pass2su(sg, 1)
                state.pop(sg, None)

            if acc is None:
                acc = psA.tile([128, D_EMB], F32, tag="acc")
                nc.vector.memset(acc[:].bitcast(F32), 0.0)
                acc_win = 0
            fl = fl_pool.tile([128, D_EMB], F32, tag="fl")
            if CONFIG["flush_engine"] == "scalar":
                nc.scalar.copy(fl[:], acc[:])
            else:
                _eng(nc, CONFIG["flush_engine"]).tensor_copy(fl[:], acc[:])
            nc.sync.dma_start(out_d[acc_win * 128:(acc_win + 1) * 128, :], fl[:])

    nc.finalize()
    return nc


def kernel(dst_input, src_attr, scalars, lin_w, lin_b, rad_w1, rad_g, rad_beta,
           rad_w2, rad_off, proj_w0, proj_b0, proj_w1, dst_index):
    dst_input = np.asarray(dst_input)
    src_attr = np.asarray(src_attr, np.float32)
    scalars = np.asarray(scalars, np.float32)
    lin_w = np.asarray(lin_w, np.float64)
    lin_b = np.asarray(lin_b, np.float64)
    rad_w1 = np.asarray(rad_w1, np.float64)
    rad_g = np.asarray(rad_g, np.float32)
    rad_beta = np.asarray(rad_beta, np.float32)
    rad_w2 = np.asarray(rad_w2, np.float64)
    rad_off = np.asarray(rad_off, np.float64)
    proj_w0 = np.asarray(proj_w0, np.float64)
    proj_b0 = np.asarray(proj_b0, np.float64)
    proj_w1 = np.asarray(proj_w1, np.float64)
    dst_index = np.asarray(dst_index)

    N = dst_input.shape[0]
    E = scalars.shape[0]
    out_dtype = dst_input.dtype
    bf16 = mybir.dt.np(BF16)

    assert np.allclose(rad_g, 1.0) and np.allclose(rad_beta, 0.0), \
        "general affine LN not supported in this build"

    # ---- host folds ----
    s0 = lin_w + lin_b                                   # [64]
    k0 = 1.0 / (math.sqrt(MUL0 + MUL1) * math.sqrt(AVG_AGG))
    k1 = 1.0 / (math.sqrt(MUL0 + 2 * MUL1) * math.sqrt(AVG_AGG))
    A0 = s0[:, None] * proj_w0[:MUL0, :]                 # [64, 64]
    A1 = s0[:, None] * proj_w1[:MUL0, :]                 # [64, 32]
    B0 = rad_w2[:, 0:64] @ A0 * k0                       # [64, 64]
    B1 = rad_w2[:, 64:128] @ A1 * k1                     # [64, 32]
    c0 = rad_off[0:64] @ A0 * k0                         # [64]
    c1 = rad_off[64:128] @ A1 * k1                       # [32]
    W1c = rad_w1 - rad_w1.mean(axis=1, keepdims=True)    # centered: h-mu fold

    # ---- edge sort and sharding ----
    NPC = (N + N_CORES - 1) // N_CORES                   # nodes per core
    NW = (NPC + WIN - 1) // WIN                          # windows per core
    order = np.argsort(dst_index, kind="stable")
    dst_sorted = dst_index[order]
    bounds = [min(k * NPC + w * WIN, N)
              for k in range(N_CORES) for w in range(NW)]
    bounds.append(N)
    bucket_edges = np.searchsorted(dst_sorted, np.asarray(bounds))
    counts = np.diff(bucket_edges).reshape(N_CORES, NW)
    # per-window tile count: max over cores (SPMD program is shared)
    tpw = np.maximum(1, -(-counts.max(axis=0) // 128))   # [NW]
    NT = int(tpw.sum())
    NT = ((NT + CONFIG["chunk_tiles"] - 1)
          // CONFIG["chunk_tiles"]) * CONFIG["chunk_tiles"]
    tile_off = np.concatenate([[0], np.cumsum(tpw)]).astype(int)
    win_of = []
    for w in range(NW):
        win_of += [w] * int(tpw[w])
    win_of += [NW - 1] * (NT - len(win_of))              # pad tiles
    win_of = tuple(win_of)
    C = NT * 128

    key = (NT, NW, win_of, tuple(sorted((k, str(v)) for k, v in CONFIG.items())))
    if key not in _PROGRAM_CACHE:
        _PROGRAM_CACHE[key] = build_program(NT, NW, list(win_of))
    nc = _PROGRAM_CACHE[key]

    # ---- per-core input arrays ----
    w1_bf = W1c.astype(np.float32).astype(bf16)
    z64 = np.zeros((64, 64)); z32 = np.zeros((64, 32))
    b0_bf = np.ascontiguousarray(np.hstack([np.vstack([B0, z64]),
                                            np.vstack([z64, B0])])
                                 .astype(np.float32)).astype(bf16)
    b1_bf = np.ascontiguousarray(np.hstack([np.vstack([B1, z32]),
                                            np.vstack([z32, B1])])
                                 .astype(np.float32)).astype(bf16)
    ident = np.eye(128, dtype=np.float32).astype(bf16)

    in_maps = []
    for k in range(N_CORES):
        xt = np.zeros((NT * 128, 64), bf16)
        oh = np.zeros((NT * 128, 128), bf16)
        aux = np.zeros((NT * 128, 4), np.float32)
        for w in range(NW):
            lo, hi = bucket_edges[k * NW + w], bucket_edges[k * NW + w + 1]
            cnt = hi - lo
            if cnt == 0:
                continue
            eidx = order[lo:hi]
            base = int(tile_off[w]) * 128
            rows = base + np.arange(cnt)
            xt[rows] = scalars[eidx].astype(bf16)
            offs = (dst_sorted[lo:hi] - (k * NPC + w * WIN)).astype(int)
            oh[rows, offs] = np.float32(1.0)
            aux[rows] = src_attr[eidx]
        m = {
            "xt": np.ascontiguousarray(xt.T),
            "oh": np.ascontiguousarray(
                oh.reshape(NT, 128, 128).transpose(1, 0, 2)
                .reshape(128, NT * 128)),
            "aux": np.ascontiguousarray(
                aux.reshape(NT, 128, 4).transpose(1, 0, 2)
                .reshape(128, NT * 4)),
            "w1c": w1_bf,
            "b0": b0_bf,
            "b1": b1_bf,
            "ident": ident,
        }
        in_maps.append(m)

    global _LAST_IN_MAPS
    _LAST_IN_MAPS = in_maps
    res = run_bass_kernel_spmd(nc, in_maps, core_ids=list(range(N_CORES)))

    # ---- host assembly ----
    out = np.zeros((N, D_EMB), np.float64)
    for k in range(N_CORES):
        rows = res.results[k]["out"]                     # [NW*128, 160]
        lo = k * NPC
        hi = min(N, (k + 1) * NPC)
        out[lo:hi] = rows[0:hi - lo]
    # device o1 layout is m-major (64 + 32*m + v); reference is 64 + 3*v + m
    blk = out[:, 64:160].reshape(N, 3, 32)
    out[:, 64:160] = blk.transpose(0, 2, 1).reshape(N, 96)

    # host-side exact corrections (rad_off and proj_b0 terms)
    if np.any(proj_b0 != 0) or np.any(c0 != 0) or np.any(c1 != 0):
        cnt = np.bincount(dst_index, minlength=N).astype(np.float64)
        suma0 = np.bincount(dst_index, weights=src_attr[:, 0].astype(np.float64),
                            minlength=N)
        out[:, 0:64] += cnt[:, None] * (proj_b0 / math.sqrt(AVG_AGG))[None, :]
        out[:, 0:64] += suma0[:, None] * c0[None, :]
        for m_ in range(3):
            sa = np.bincount(dst_index,
                             weights=src_attr[:, 1 + m_].astype(np.float64),
                             minlength=N)
            out[:, 64 + m_::3][:, 0:32] += sa[:, None] * c1[None, :]

    return out.astype(out_dtype)
